# revision 13
# baseline (speedup 1.0000x reference)
"""CantorAttention Trainium2 kernel — block-sparse routed attention.

Problem (hardcoded): B=2, S=2048, DIM=512, H=8 heads, D=64, K=64 routes.
  qkv = x @ w_qkv + b_qkv ; per-head softmax attention over routes[q, :] ;
  out = attn_out @ w_out + b_out.

Strategy (8 cores): shard batch x head-pairs. Core i handles batch i//4 and
heads (2*(i%4), 2*(i%4)+1).

Sparsity exploit: routes are distinct per query (binary mask).  A single
token permutation (iterated sort by mean routed-neighbour index — derived
from the routes alone) makes the [S, S] route mask block-sparse: each
256-query block touches only a few 128-key tiles.  The host builds that
schedule and compiles a kernel specialized to it; attention runs dense only
on the touched (key-tile, query-block) pairs with an additive {0, -200}
mask folded into the score PSUM via an identity matmul.

Algebraic simplifications:
  - K-bias dropped: (q+bq).(k+bk) differs from (q+bq).k by a per-query
    constant -> cancels in softmax.
  - V-bias folded into the output bias on the host (softmax weights sum
    to 1), so V = x @ wv with no bias and the host adds
    b_out + b_qkv[2*DIM:] @ w_out once.
  - Denominators ride along in the AV matmul via a ones column appended to
    V (output row 64), then one reciprocal + a ones-row broadcast matmul
    replicates 1/den across the head's 64 partitions for normalization.

Both heads of a core run concurrently in the QK matmuls via PE row tiling
(contraction 64 each, tile_position rows 0-63 / 64-127).
"""

import numpy as np
import ml_dtypes

import concourse.bass as bass
import concourse.bacc as bacc
import concourse.mybir as mybir
import concourse.tile as tile
from concourse.bass_utils import run_bass_kernel_spmd
from concourse.masks import make_identity

BF16 = mybir.dt.bfloat16
F32 = mybir.dt.float32
F8 = mybir.dt.float8e4
NPBF16 = ml_dtypes.bfloat16
NPF8 = ml_dtypes.float8_e4m3

B = 2
S = 2048
DIM = 512
H = 8
D = 64
KR = 64
SCALE = 0.125

P = 128
NC4 = DIM // P    # 4 contraction chunks
QB = 256          # query block
NQB = S // QB     # 8 query blocks
NKT = S // P      # 16 key tiles
VW = D + 1        # v tile width incl ones column
MASKNEG = -192.0  # exact in fp8 e4m3 (TRN max-normal 240); exp(-192) == 0

_CACHE = {}
DBG = set()  # debug: "nonorm", "expbank", "norowtile"


def _token_order(routes):
    """Permutation clustering tokens so each query block touches few key
    tiles.  Iterated argsort by mean routed-neighbour position; generic
    (no Cantor assumption) and a no-op perf-wise for random routes."""
    n = routes.shape[0]
    order = np.argsort(routes.mean(axis=1), kind="stable")
    for _ in range(3):
        inv = np.empty(n, np.int64)
        inv[order] = np.arange(n)
        m = inv[routes].mean(axis=1)
        order = order[np.argsort(m[order], kind="stable")]
    return order


def _schedule(routes):
    routes = np.asarray(routes, np.int64)
    order = _token_order(routes)
    inv = np.empty(S, np.int64)
    inv[order] = np.arange(S)
    rk = inv[routes][order]          # [S, K] both sides permuted
    tiles = rk // P
    sched = tuple(
        tuple(sorted(set(tiles[qb * QB:(qb + 1) * QB].ravel().tolist())))
        for qb in range(NQB)
    )
    return order, rk, sched


def build_nc(sched):
    key = (sched, tuple(sorted(DBG)))
    if key in _CACHE:
        return _CACHE[key]
    npair = sum(len(k) for k in sched)
    nc = bacc.Bacc(
        "TRN2",
        target_bir_lowering=False,
        debug=False,
        num_devices=8,
    )

    xt_d = nc.dram_tensor("xt", [P, NC4 * S], BF16, kind="ExternalInput").ap()
    wq_d = nc.dram_tensor("wq", [P, NC4 * P], BF16, kind="ExternalInput").ap()
    wk_d = nc.dram_tensor("wk", [P, NC4 * P], BF16, kind="ExternalInput").ap()
    wv_d = nc.dram_tensor("wv", [P, NC4 * P], BF16, kind="ExternalInput").ap()
    bq_d = nc.dram_tensor("bq", [P, 1], F32, kind="ExternalInput").ap()
    msk_d = nc.dram_tensor("msk", [P, npair * QB], F8, kind="ExternalInput").ap()
    wo_d = nc.dram_tensor("wo", [P, DIM], BF16, kind="ExternalInput").ap()
    out_d = nc.dram_tensor("out", [S, DIM], BF16, kind="ExternalOutput").ap()
    if "dump" in DBG:
        dq_d = nc.dram_tensor("dq", [P, S], F32, kind="ExternalOutput").ap()
        dk_d = nc.dram_tensor("dk", [P, S], F32, kind="ExternalOutput").ap()
        dv0_d = nc.dram_tensor("dv0", [P, NKT * VW], F32, kind="ExternalOutput").ap()
        dv1_d = nc.dram_tensor("dv1", [P, NKT * VW], F32, kind="ExternalOutput").ap()
        don_d = nc.dram_tensor("don", [P, S], F32, kind="ExternalOutput").ap()
        do_d = nc.dram_tensor("do_", [P, NQB * 2 * QB], F32, kind="ExternalOutput").ap()
        dpm_d = nc.dram_tensor("dpm", [P, NQB * 4 * QB], F32, kind="ExternalOutput").ap()

    qb_off = []
    off = 0
    for kts in sched:
        qb_off.append(off)
        off += len(kts) * QB

    with tile.TileContext(nc) as tc:
        with tc.tile_pool(name="persist", bufs=1) as pp:
            ident = pp.tile([P, P], BF16, tag="ident")
            make_identity(nc, ident[:])
            idf8 = pp.tile([P, P], F8, tag="idf8")
            nc.scalar.copy(out=idf8[:], in_=ident[:])
            onesr = pp.tile([1, P], F32, tag="onesr")
            nc.vector.memset(onesr[:], 1.0)

            # DMA issue order = phase-A consumption order; xt comes in four
            # 3D-AP strided loads (one per 512-query slab across all 4
            # contraction chunks); mask rides the idle GpSimd SWDGE queues.
            w_sb = {}
            for name, wd in (("k", wk_d), ("q", wq_d), ("v", wv_d)):
                w_sb[name] = pp.tile([P, NC4 * P], BF16, tag=f"w{name}",
                                     name=f"w{name}_sb")
            xt_sb = pp.tile([P, NC4 * S], BF16, tag="xt", name="xt_sb")
            xt_dv = xt_d.rearrange("p (c s) -> p c s", c=NC4)
            xt_sv = xt_sb[:].rearrange("p (c s) -> p c s", c=NC4)
            bq_sb = pp.tile([P, 1], F32, tag="bq")
            wo_sb = pp.tile([P, DIM], BF16, tag="wo")
            nc.sync.dma_start(out=w_sb["k"][:], in_=wk_d[:, :])
            nc.sync.dma_start(
                out=xt_sv[:, :, 0:512], in_=xt_dv[:, :, 0:512])
            nc.sync.dma_start(out=w_sb["q"][:], in_=wq_d[:, :])
            nc.sync.dma_start(
                out=xt_sv[:, :, 512:1024], in_=xt_dv[:, :, 512:1024])
            nc.sync.dma_start(out=w_sb["v"][:], in_=wv_d[:, :])
            nc.sync.dma_start(
                out=xt_sv[:, :, 1024:1536], in_=xt_dv[:, :, 1024:1536])
            nc.sync.dma_start(out=bq_sb[:], in_=bq_d[:, :])
            nc.sync.dma_start(out=wo_sb[:], in_=wo_d[:, :])
            nc.sync.dma_start(
                out=xt_sv[:, :, 1536:2048], in_=xt_dv[:, :, 1536:2048])
            msk_sb = pp.tile([P, npair * QB], F8, tag="msk", name="msk_sb")
            for g3 in range(0, NQB, 3):
                o0 = qb_off[g3]
                end = (qb_off[g3 + 3] if g3 + 3 < NQB
                       else npair * QB)
                nc.gpsimd.dma_start(
                    out=msk_sb[:, o0:end], in_=msk_d[:, o0:end]
                )

            qT = pp.tile([P, S], BF16, tag="qT")
            kT = pp.tile([P, S], BF16, tag="kT")
            v0 = pp.tile([P, NKT * VW], BF16, tag="v0")
            v1 = pp.tile([P, NKT * VW], BF16, tag="v1")
            nc.vector.memset(v0[:], 1.0)
            nc.vector.memset(v1[:], 1.0)
            on_sb = pp.tile([P, S], BF16, tag="on")

            # ---- Phase A: projections ----
            QC = 512
            with tc.tile_pool(name="pa", bufs=3, space="PSUM") as pa:
                for qc in range(NC4):          # kT = Wk^T X^T  (no bias)
                    ps = pa.tile([P, QC], F32, tag="qkps", name="kps")
                    for c in range(NC4):
                        nc.tensor.matmul(
                            ps[:],
                            lhsT=w_sb["k"][:, c * P:(c + 1) * P],
                            rhs=xt_sb[:, c * S + qc * QC: c * S + qc * QC + QC],
                            start=(c == 0),
                            stop=(c == NC4 - 1),
                        )
                    if qc % 2 == 0:
                        nc.scalar.copy(out=kT[:, qc * QC:(qc + 1) * QC], in_=ps[:])
                    else:
                        nc.vector.tensor_copy(
                            out=kT[:, qc * QC:(qc + 1) * QC], in_=ps[:]
                        )
                for qc in range(NC4):          # qT = Wq'^T X^T + bq'  (scaled)
                    ps = pa.tile([P, QC], F32, tag="qkps", name="qps")
                    for c in range(NC4):
                        nc.tensor.matmul(
                            ps[:],
                            lhsT=w_sb["q"][:, c * P:(c + 1) * P],
                            rhs=xt_sb[:, c * S + qc * QC: c * S + qc * QC + QC],
                            start=(c == 0),
                            stop=(c == NC4 - 1),
                        )
                    nc.vector.tensor_tensor(
                        out=qT[:, qc * QC:(qc + 1) * QC],
                        in0=ps[:],
                        in1=bq_sb[:].to_broadcast([P, QC]),
                        op=mybir.AluOpType.add,
                    )
                for kt in range(NKT):          # V direct form [tokens, dims]
                    vps = pa.tile([P, P], F32, tag="vps", name="vps")
                    for c in range(NC4):
                        nc.tensor.matmul(
                            vps[:],
                            lhsT=xt_sb[:, c * S + kt * P: c * S + kt * P + P],
                            rhs=w_sb["v"][:, c * P:(c + 1) * P],
                            start=(c == 0),
                            stop=(c == NC4 - 1),
                        )
                    d0 = kt * VW
                    if kt % 2 == 0:
                        nc.vector.tensor_copy(out=v0[:, d0:d0 + D], in_=vps[:, 0:D])
                        nc.scalar.copy(out=v1[:, d0:d0 + D], in_=vps[:, D:2 * D])
                    else:
                        nc.scalar.copy(out=v0[:, d0:d0 + D], in_=vps[:, 0:D])
                        nc.vector.tensor_copy(out=v1[:, d0:d0 + D], in_=vps[:, D:2 * D])

            # ---- Phase B: block-sparse attention + normalize + project ----
            with tc.tile_pool(name="ps_s", bufs=2, space="PSUM") as sp, \
                 tc.tile_pool(name="ps_o", bufs=2, space="PSUM") as opool, \
                 tc.tile_pool(name="ps_r", bufs=1, space="PSUM") as rp, \
                 tc.tile_pool(name="ps_pr", bufs=1, space="PSUM") as prp, \
                 tc.tile_pool(name="pb", bufs=3) as pb, \
                 tc.tile_pool(name="fin", bufs=3) as fsb:
                for qb in range(NQB):
                    kts = sched[qb]
                    nkt = len(kts)
                    qs = slice(qb * QB, (qb + 1) * QB)
                    o = opool.tile([P, 2 * QB], F32, tag="o", name="o")
                    groups = [kts[i:i + 2] for i in range(0, nkt, 2)]
                    pi = 0
                    for g in groups:
                        # s spans 2 banks: bank A (cols 0:2QB) holds h0
                        # scores for the group's pairs, bank B (2QB:4QB)
                        # holds h1.  Row-group-64 matmuls must not share a
                        # bank with row-group-0 ones (HW crash), so heads
                        # get separate banks; ident mask matmuls (full 128)
                        # may share with either.
                        ng = len(g)
                        s = sp.tile([P, 4 * QB], F32, tag="s", name="s")
                        pmt = pb.tile([P, 4 * QB], BF16, tag="pm", name="pm")
                        for jj, kt in enumerate(g):
                            a0 = jj * QB              # h0 region (bank A)
                            a1 = 2 * QB + jj * QB     # h1 region (bank B)
                            moff = qb_off[qb] + (pi + jj) * QB
                            if "norowtile" in DBG:
                                nc.tensor.matmul(
                                    s[:, a0:a0 + QB],
                                    lhsT=kT[:, kt * P:(kt + 1) * P],
                                    rhs=qT[:, qs],
                                    start=(jj == 0), stop=False,
                                )
                                nc.tensor.matmul(
                                    s[:, a1:a1 + QB],
                                    lhsT=kT[:, kt * P:(kt + 1) * P],
                                    rhs=qT[:, qs],
                                    start=(jj == 0), stop=False,
                                )
                            else:
                                nc.tensor.matmul(
                                    s[:, a0:a0 + QB],
                                    lhsT=kT[0:D, kt * P:(kt + 1) * P],
                                    rhs=qT[0:D, qs],
                                    start=(jj == 0), stop=False,
                                )
                                nc.tensor.matmul(
                                    s[:, a1:a1 + QB],
                                    lhsT=kT[D:P, kt * P:(kt + 1) * P],
                                    rhs=qT[D:P, qs],
                                    start=(jj == 0), stop=False,
                                )
                            nc.tensor.matmul(
                                s[:, a0:a0 + QB],
                                lhsT=idf8[:],
                                rhs=msk_sb[:, moff:moff + QB],
                                start=False, stop=(jj == ng - 1),
                            )
                            nc.tensor.matmul(
                                s[:, a1:a1 + QB],
                                lhsT=idf8[:],
                                rhs=msk_sb[:, moff:moff + QB],
                                start=False, stop=(jj == ng - 1),
                            )
                        if ng == 2:
                            nc.scalar.activation(
                                pmt[:], s[:], mybir.ActivationFunctionType.Exp
                            )
                        else:
                            nc.scalar.activation(
                                pmt[:, 0:QB], s[:, 0:QB],
                                mybir.ActivationFunctionType.Exp
                            )
                            nc.scalar.activation(
                                pmt[:, 2 * QB:3 * QB], s[:, 2 * QB:3 * QB],
                                mybir.ActivationFunctionType.Exp
                            )
                        for jj, kt in enumerate(g):
                            j = pi + jj
                            a0 = jj * QB
                            a1 = 2 * QB + jj * QB
                            nc.tensor.matmul(
                                o[0:VW, 0:QB],
                                lhsT=v0[:, kt * VW:(kt + 1) * VW],
                                rhs=pmt[:, a0:a0 + QB],
                                start=(j == 0), stop=False,
                            )
                            nc.tensor.matmul(
                                o[0:VW, QB:2 * QB],
                                lhsT=v1[:, kt * VW:(kt + 1) * VW],
                                rhs=pmt[:, a1:a1 + QB],
                                start=False, stop=(j == nkt - 1),
                            )
                        pi += len(g)
                        if "dump" in DBG and pi == len(g):
                            dt_ = pb.tile([P, 4 * QB], F32, tag="dmp", name="dmp")
                            nc.scalar.copy(out=dt_[:], in_=pmt[:])
                            nc.sync.dma_start(
                                out=dpm_d[:, qb * 4 * QB: (qb * 4 + 4) * QB],
                                in_=dt_[:])

                    if "dump" in DBG:
                        dt2 = pb.tile([P, 2 * QB], F32, tag="dmp2", name="dmp2")
                        nc.vector.tensor_copy(out=dt2[:], in_=o[:])
                        nc.sync.dma_start(
                            out=do_d[:, qb * 2 * QB:(qb + 1) * 2 * QB], in_=dt2[:])
                    if "nonorm" in DBG:
                        nc.vector.tensor_copy(out=on_sb[0:D, qs], in_=o[0:D, 0:QB])
                        nc.vector.tensor_copy(out=on_sb[D:P, qs], in_=o[0:D, QB:2 * QB])
                    else:
                        den_sb = pb.tile([1, 2 * QB], F32, tag="den", name="den_sb")
                        # custom-DVE recip misreads PSUM at partition base 64;
                        # stage the den row through SBUF first.
                        nc.scalar.copy(out=den_sb[:], in_=o[D:D + 1, :])
                        rd = pb.tile([1, 2 * QB], F32, tag="rd", name="rd")
                        nc.vector.reciprocal_approx_fast(out=rd[:], in_=den_sb[:])
                        rep = rp.tile([P, 2 * QB], F32, tag="rep", name="rep")
                        nc.tensor.matmul(
                            rep[:, 0:QB], lhsT=onesr[:], rhs=rd[0:1, 0:QB],
                            start=True, stop=False,
                        )
                        nc.tensor.matmul(
                            rep[:, QB:2 * QB], lhsT=onesr[:], rhs=rd[0:1, QB:2 * QB],
                            start=False, stop=True,
                        )
                        rep_sb = pb.tile([P, 2 * QB], BF16, tag="repsb", name="rep_sb")
                        nc.vector.tensor_copy(out=rep_sb[:], in_=rep[:])
                        nc.vector.tensor_tensor(
                            out=on_sb[0:D, qs], in0=o[0:D, 0:QB],
                            in1=rep_sb[0:D, 0:QB],
                            op=mybir.AluOpType.mult,
                        )
                        nc.vector.tensor_tensor(
                            out=on_sb[D:P, qs], in0=o[0:D, QB:2 * QB],
                            in1=rep_sb[D:P, QB:2 * QB],
                            op=mybir.AluOpType.mult,
                        )
                    ob = fsb.tile([P, 2 * DIM], BF16, tag="ob", name="ob")
                    for t in range(QB // P):
                        qt = qb * (QB // P) + t
                        pr = prp.tile([P, DIM], F32, tag="pr", name="pr")
                        nc.tensor.matmul(
                            pr[:],
                            lhsT=on_sb[:, qt * P:(qt + 1) * P],
                            rhs=wo_sb[:],
                            start=True, stop=True,
                        )
                        if t == 0:
                            nc.scalar.copy(out=ob[:, 0:DIM], in_=pr[:])
                        else:
                            nc.vector.tensor_copy(out=ob[:, DIM:2 * DIM], in_=pr[:])
                    nc.sync.dma_start(
                        out=out_d[qb * QB:(qb + 1) * QB, :].rearrange(
                            "(t p) d -> p t d", p=P),
                        in_=ob[:].rearrange("p (t d) -> p t d", t=2),
                    )

                if "dump" in DBG:
                    for nm, src_t, dst in (("dq", qT, dq_d), ("dk", kT, dk_d),
                                           ("dv0", v0, dv0_d), ("dv1", v1, dv1_d),
                                           ("don", on_sb, don_d)):
                        w = src_t.shape[1]
                        for c0 in range(0, w, 2048):
                            cw = min(2048, w - c0)
                            dt3 = fsb.tile([P, 2048], F32, tag="dmp3", name="dmp3")
                            nc.vector.tensor_copy(out=dt3[:, 0:cw], in_=src_t[:, c0:c0 + cw])
                            nc.sync.dma_start(out=dst[:, c0:c0 + cw], in_=dt3[:, 0:cw])

    nc.compile()
    _CACHE[key] = nc
    return nc


def _pack(a):
    # [n*128, X] -> [128, n*X]
    n = a.shape[0] // P
    return np.ascontiguousarray(
        a.reshape(n, P, a.shape[1]).transpose(1, 0, 2).reshape(P, -1))


def make_in_maps(x, routes, w_qkv, b_qkv, w_out, order, rk, sched):
    x = np.asarray(x, np.float32)
    w_qkv = np.asarray(w_qkv, np.float32)
    b_qkv = np.asarray(b_qkv, np.float32)
    w_out = np.asarray(w_out, np.float32)

    Cm = np.zeros((S, S), bool)
    Cm[np.arange(S)[:, None], rk] = True          # permuted [q', k']
    cols = []
    for qb, kts in enumerate(sched):
        blk = Cm[qb * QB:(qb + 1) * QB]
        for kt in kts:
            sub = blk[:, kt * P:(kt + 1) * P]     # [256 q, 128 k]
            cols.append(np.where(sub.T, 0.0, MASKNEG))
    msk = np.concatenate(cols, axis=1).astype(NPF8)

    xt = [
        _pack(np.ascontiguousarray(x[b].T[:, order])).astype(NPBF16)
        for b in range(B)
    ]

    in_maps = []
    for core in range(8):
        b = core // 4
        hp = core % 4
        col = hp * P
        wq = _pack(w_qkv[:, col:col + P] * SCALE).astype(NPBF16)
        wk = _pack(w_qkv[:, DIM + col:DIM + col + P]).astype(NPBF16)
        wv = _pack(w_qkv[:, 2 * DIM + col:2 * DIM + col + P]).astype(NPBF16)
        bq = (b_qkv[col:col + P] * SCALE).astype(np.float32).reshape(P, 1)
        wo = np.ascontiguousarray(w_out[col:col + P, :]).astype(NPBF16)
        in_maps.append(dict(
            xt=xt[b], wq=wq, wk=wk, wv=wv, bq=bq, msk=msk, wo=wo,
        ))
    return in_maps


def run(inputs, trace=False, trace_cores=None):
    routes = np.asarray(inputs["routes"])
    order, rk, sched = _schedule(routes)
    nc = build_nc(sched)
    in_maps = make_in_maps(
        inputs["x"], routes, inputs["w_qkv"], inputs["b_qkv"],
        inputs["w_out"], order, rk, sched,
    )
    res = run_bass_kernel_spmd(
        nc, in_maps, list(range(8)), trace=trace, trace_cores=trace_cores,
    )
    b_qkv = np.asarray(inputs["b_qkv"], np.float32)
    w_out = np.asarray(inputs["w_out"], np.float32)
    bias = np.asarray(inputs["b_out"], np.float32) + b_qkv[2 * DIM:] @ w_out
    final = np.zeros((B, S, DIM), np.float32)
    for core in range(8):
        final[core // 4][order] += np.asarray(res.results[core]["out"], np.float32)
    final += bias[None, None, :]
    return final, res


def kernel(**inputs):
    final, _ = run(inputs, trace=False)
    return final


# revision 14
# speedup vs baseline: 1.0326x; 1.0326x over previous
"""CantorAttention Trainium2 kernel — block-sparse routed attention.

Problem (hardcoded): B=2, S=2048, DIM=512, H=8 heads, D=64, K=64 routes.
  qkv = x @ w_qkv + b_qkv ; per-head softmax attention over routes[q, :] ;
  out = attn_out @ w_out + b_out.

Strategy (8 cores): shard batch x head-pairs. Core i handles batch i//4 and
heads (2*(i%4), 2*(i%4)+1).

Sparsity exploit: routes are distinct per query (binary mask).  A single
token permutation (iterated sort by mean routed-neighbour index — derived
from the routes alone) makes the [S, S] route mask block-sparse: each
256-query block touches only a few 128-key tiles.  The host builds that
schedule and compiles a kernel specialized to it; attention runs dense only
on the touched (key-tile, query-block) pairs with an additive {0, -192}
mask folded into the score PSUM via an fp8 identity matmul.

Algebraic simplifications:
  - K-bias dropped: (q+bq).(k+bk) differs from (q+bq).k by a per-query
    constant -> cancels in softmax.
  - V-bias folded into the output bias on the host (softmax weights sum
    to 1), so V = x @ wv with no bias and the host adds
    b_out + b_qkv[2*DIM:] @ w_out once.
  - Denominators ride along in the AV matmul via a ones column appended to
    V (output row 64), then one reciprocal + ones-row broadcast matmuls
    replicate 1/den across each head's 64 partitions for normalization.

Hardware notes baked in:
  - Both heads run concurrently in QK via PE row tiling (contraction 64,
    rows 0-63 / 64-127) but their outputs go to SEPARATE PSUM banks —
    mixing two row-tiled matmuls in one bank crashes the device.  Each
    bank gets exactly one start=True (clears whole-bank has_written);
    every later write overwrites-where-unset / accumulates-where-set.
  - The custom-DVE reciprocal misreads PSUM at partition base 64, so the
    denominator row is staged through SBUF first.
  - Tile tracks dependencies per-tile, so inputs are split into many
    small tiles (x^T slabs, per-chunk q^T/k^T, per-key-tile V, per-group
    masks, per-block outputs) to let phases overlap.
"""

import numpy as np
import ml_dtypes

import concourse.bass as bass
import concourse.bacc as bacc
import concourse.mybir as mybir
import concourse.tile as tile
from concourse.bass_utils import run_bass_kernel_spmd
from concourse.masks import make_identity

BF16 = mybir.dt.bfloat16
F32 = mybir.dt.float32
F8 = mybir.dt.float8e4
NPBF16 = ml_dtypes.bfloat16
NPF8 = ml_dtypes.float8_e4m3

B = 2
S = 2048
DIM = 512
H = 8
D = 64
KR = 64
SCALE = 0.125

P = 128
NC4 = DIM // P    # 4 contraction chunks
QC = 512          # projection slab width
QB = 256          # query block
NQB = S // QB     # 8 query blocks
NKT = S // P      # 16 key tiles
VW = D + 1        # v tile width incl ones column
MASKNEG = -192.0  # exact in fp8 e4m3 (TRN max normal 240); exp(-192) == 0

_CACHE = {}


def _token_order(routes):
    """Permutation clustering tokens so each query block touches few key
    tiles.  Iterated argsort by mean routed-neighbour position; generic
    (no Cantor assumption)."""
    n = routes.shape[0]
    order = np.argsort(routes.mean(axis=1), kind="stable")
    for _ in range(3):
        inv = np.empty(n, np.int64)
        inv[order] = np.arange(n)
        m = inv[routes].mean(axis=1)
        order = order[np.argsort(m[order], kind="stable")]
    return order


def _schedule(routes):
    routes = np.asarray(routes, np.int64)
    order = _token_order(routes)
    inv = np.empty(S, np.int64)
    inv[order] = np.arange(S)
    rk = inv[routes][order]          # [S, K] both sides permuted
    tiles = rk // P
    sched = tuple(
        tuple(sorted(set(tiles[qb * QB:(qb + 1) * QB].ravel().tolist())))
        for qb in range(NQB)
    )
    return order, rk, sched


def build_nc(sched):
    if sched in _CACHE:
        return _CACHE[sched]
    npair = sum(len(k) for k in sched)
    nc = bacc.Bacc(
        "TRN2",
        target_bir_lowering=False,
        debug=False,
        num_devices=8,
    )

    xt_d = nc.dram_tensor("xt", [P, NC4 * S], BF16, kind="ExternalInput").ap()
    wq_d = nc.dram_tensor("wq", [P, NC4 * P], BF16, kind="ExternalInput").ap()
    wk_d = nc.dram_tensor("wk", [P, NC4 * P], BF16, kind="ExternalInput").ap()
    wv_d = nc.dram_tensor("wv", [P, NC4 * P], BF16, kind="ExternalInput").ap()
    bq_d = nc.dram_tensor("bq", [P, 1], F32, kind="ExternalInput").ap()
    msk_d = nc.dram_tensor("msk", [P, npair * QB], F8, kind="ExternalInput").ap()
    wo_d = nc.dram_tensor("wo", [P, DIM], BF16, kind="ExternalInput").ap()
    out_d = nc.dram_tensor("out", [S, DIM], BF16, kind="ExternalOutput").ap()

    qb_off = []
    off = 0
    for kts in sched:
        qb_off.append(off)
        off += len(kts) * QB

    # V tiles ordered by first use across query blocks
    v_order = []
    for kts in sched:
        for kt in kts:
            if kt not in v_order:
                v_order.append(kt)
    for kt in range(NKT):
        if kt not in v_order:
            v_order.append(kt)

    # mask tile DMA grouping
    mgrp = [list(range(0, 3)), list(range(3, 6)), list(range(6, NQB))]

    with tile.TileContext(nc) as tc:
        with tc.tile_pool(name="persist", bufs=1) as pp:
            ident = pp.tile([P, P], BF16, tag="ident")
            make_identity(nc, ident[:])
            idf8 = pp.tile([P, P], F8, tag="idf8")
            nc.scalar.copy(out=idf8[:], in_=ident[:])
            onesr = pp.tile([1, P], F32, tag="onesr")
            nc.vector.memset(onesr[:], 1.0)

            # --- input DMAs, consumption-ordered, fine-grained tiles ---
            w_sb = {}
            for name in ("k", "q", "v"):
                w_sb[name] = pp.tile([P, NC4 * P], BF16, tag=f"w{name}",
                                     name=f"w{name}_sb")
            xt_dv = xt_d.rearrange("p (c s) -> p c s", c=NC4)
            xts = [pp.tile([P, NC4 * QC], BF16, tag=f"xt{qc}",
                           name=f"xt{qc}") for qc in range(NC4)]
            bq_sb = pp.tile([P, 1], F32, tag="bq")
            wo_sb = pp.tile([P, DIM], BF16, tag="wo")
            msk_sb = {}
            for gi, grp in enumerate(mgrp):
                w = sum(len(sched[qb]) for qb in grp) * QB
                msk_sb[gi] = pp.tile([P, w], F8, tag=f"msk{gi}",
                                     name=f"msk{gi}")

            nc.sync.dma_start(out=w_sb["k"][:], in_=wk_d[:, :])
            nc.sync.dma_start(
                out=xts[0][:].rearrange("p (c s) -> p c s", c=NC4),
                in_=xt_dv[:, :, 0:QC])
            nc.sync.dma_start(out=w_sb["q"][:], in_=wq_d[:, :])
            nc.sync.dma_start(
                out=msk_sb[0][:],
                in_=msk_d[:, qb_off[0]:qb_off[0] + msk_sb[0].shape[1]])
            nc.sync.dma_start(
                out=xts[1][:].rearrange("p (c s) -> p c s", c=NC4),
                in_=xt_dv[:, :, QC:2 * QC])
            nc.sync.dma_start(out=w_sb["v"][:], in_=wv_d[:, :])
            nc.sync.dma_start(out=bq_sb[:], in_=bq_d[:, :])
            nc.sync.dma_start(
                out=xts[2][:].rearrange("p (c s) -> p c s", c=NC4),
                in_=xt_dv[:, :, 2 * QC:3 * QC])
            nc.sync.dma_start(out=wo_sb[:], in_=wo_d[:, :])
            nc.sync.dma_start(
                out=xts[3][:].rearrange("p (c s) -> p c s", c=NC4),
                in_=xt_dv[:, :, 3 * QC:4 * QC])
            for gi in (1, 2):
                o0 = qb_off[mgrp[gi][0]]
                nc.gpsimd.dma_start(
                    out=msk_sb[gi][:], in_=msk_d[:, o0:o0 + msk_sb[gi].shape[1]])

            def mslice(qb, pi):
                gi = 0 if qb < 3 else (1 if qb < 6 else 2)
                o0 = qb_off[qb] - qb_off[mgrp[gi][0]] + pi * QB
                return msk_sb[gi][:, o0:o0 + QB]

            kTs = [pp.tile([P, QC], BF16, tag=f"kT{i}", name=f"kT{i}")
                   for i in range(NC4)]
            qTs = [pp.tile([P, QC], BF16, tag=f"qT{i}", name=f"qT{i}")
                   for i in range(NC4)]
            v0t = {kt: pp.tile([P, VW], BF16, tag=f"v0_{kt}", name=f"v0_{kt}")
                   for kt in range(NKT)}
            v1t = {kt: pp.tile([P, VW], BF16, tag=f"v1_{kt}", name=f"v1_{kt}")
                   for kt in range(NKT)}
            for kt in range(NKT):
                nc.vector.memset(v0t[kt][:, D:VW], 1.0)
                nc.vector.memset(v1t[kt][:, D:VW], 1.0)
            ons = [pp.tile([P, QB], BF16, tag=f"on{qb}", name=f"on{qb}")
                   for qb in range(NQB)]

            def kslice(kt, rows):
                return kTs[kt // NC4][rows, (kt % NC4) * P:(kt % NC4 + 1) * P]

            def qslice(qb, rows):
                return qTs[qb // 2][rows, (qb % 2) * QB:(qb % 2 + 1) * QB]

            # ---- Phase A: projections ----
            with tc.tile_pool(name="pa", bufs=3, space="PSUM") as pa:
                for qc in range(NC4):          # kT = Wk^T X^T  (no bias)
                    ps = pa.tile([P, QC], F32, tag="qkps", name="kps")
                    for c in range(NC4):
                        nc.tensor.matmul(
                            ps[:],
                            lhsT=w_sb["k"][:, c * P:(c + 1) * P],
                            rhs=xts[qc][:, c * QC:(c + 1) * QC],
                            start=(c == 0),
                            stop=(c == NC4 - 1),
                        )
                    if qc % 2 == 0:
                        nc.scalar.copy(out=kTs[qc][:], in_=ps[:])
                    else:
                        nc.vector.tensor_copy(out=kTs[qc][:], in_=ps[:])
                for qc in range(NC4):          # qT = Wq'^T X^T + bq' (scaled)
                    ps = pa.tile([P, QC], F32, tag="qkps", name="qps")
                    for c in range(NC4):
                        nc.tensor.matmul(
                            ps[:],
                            lhsT=w_sb["q"][:, c * P:(c + 1) * P],
                            rhs=xts[qc][:, c * QC:(c + 1) * QC],
                            start=(c == 0),
                            stop=(c == NC4 - 1),
                        )
                    nc.vector.tensor_tensor(
                        out=qTs[qc][:],
                        in0=ps[:],
                        in1=bq_sb[:].to_broadcast([P, QC]),
                        op=mybir.AluOpType.add,
                    )
                for vi, kt in enumerate(v_order):   # V direct [tokens, dims]
                    vps = pa.tile([P, P], F32, tag="vps", name="vps")
                    for c in range(NC4):
                        nc.tensor.matmul(
                            vps[:],
                            lhsT=xts[kt // NC4][:, c * QC + (kt % NC4) * P:
                                                c * QC + (kt % NC4) * P + P],
                            rhs=w_sb["v"][:, c * P:(c + 1) * P],
                            start=(c == 0),
                            stop=(c == NC4 - 1),
                        )
                    if vi % 2 == 0:
                        nc.vector.tensor_copy(out=v0t[kt][:, 0:D], in_=vps[:, 0:D])
                        nc.scalar.copy(out=v1t[kt][:, 0:D], in_=vps[:, D:2 * D])
                    else:
                        nc.scalar.copy(out=v0t[kt][:, 0:D], in_=vps[:, 0:D])
                        nc.vector.tensor_copy(out=v1t[kt][:, 0:D], in_=vps[:, D:2 * D])

            # ---- Phase B: block-sparse attention + normalize + project ----
            with tc.tile_pool(name="ps_s", bufs=2, space="PSUM") as sp, \
                 tc.tile_pool(name="ps_o", bufs=2, space="PSUM") as opool, \
                 tc.tile_pool(name="ps_m", bufs=2, space="PSUM") as mp, \
                 tc.tile_pool(name="pb", bufs=3) as pb, \
                 tc.tile_pool(name="fin", bufs=3) as fsb:
                for qb in range(NQB):
                    kts = sched[qb]
                    nkt = len(kts)
                    o = opool.tile([P, 2 * QB], F32, tag="o", name="o")
                    groups = [kts[i:i + 2] for i in range(0, nkt, 2)]
                    pi = 0
                    for g in groups:
                        # s spans 2 banks: bank A (cols 0:2QB) holds the
                        # group's h0 scores, bank B (2QB:4QB) holds h1.
                        ng = len(g)
                        s = sp.tile([P, 4 * QB], F32, tag="s", name="s")
                        pmt = pb.tile([P, 4 * QB], BF16, tag="pm", name="pm")
                        for jj, kt in enumerate(g):
                            a0 = jj * QB              # h0 region (bank A)
                            a1 = 2 * QB + jj * QB     # h1 region (bank B)
                            nc.tensor.matmul(
                                s[:, a0:a0 + QB],
                                lhsT=kslice(kt, slice(0, D)),
                                rhs=qslice(qb, slice(0, D)),
                                start=(jj == 0), stop=False,
                            )
                            nc.tensor.matmul(
                                s[:, a1:a1 + QB],
                                lhsT=kslice(kt, slice(D, P)),
                                rhs=qslice(qb, slice(D, P)),
                                start=(jj == 0), stop=False,
                            )
                            nc.tensor.matmul(
                                s[:, a0:a0 + QB],
                                lhsT=idf8[:],
                                rhs=mslice(qb, pi + jj),
                                start=False, stop=(jj == ng - 1),
                            )
                            nc.tensor.matmul(
                                s[:, a1:a1 + QB],
                                lhsT=idf8[:],
                                rhs=mslice(qb, pi + jj),
                                start=False, stop=(jj == ng - 1),
                            )
                        if ng == 2:
                            nc.scalar.activation(
                                pmt[:], s[:], mybir.ActivationFunctionType.Exp
                            )
                        else:
                            nc.scalar.activation(
                                pmt[:, 0:QB], s[:, 0:QB],
                                mybir.ActivationFunctionType.Exp
                            )
                            nc.scalar.activation(
                                pmt[:, 2 * QB:3 * QB], s[:, 2 * QB:3 * QB],
                                mybir.ActivationFunctionType.Exp
                            )
                        for jj, kt in enumerate(g):
                            j = pi + jj
                            a0 = jj * QB
                            a1 = 2 * QB + jj * QB
                            nc.tensor.matmul(
                                o[0:VW, 0:QB],
                                lhsT=v0t[kt][:],
                                rhs=pmt[:, a0:a0 + QB],
                                start=(j == 0), stop=False,
                            )
                            nc.tensor.matmul(
                                o[0:VW, QB:2 * QB],
                                lhsT=v1t[kt][:],
                                rhs=pmt[:, a1:a1 + QB],
                                start=False, stop=(j == nkt - 1),
                            )
                        pi += len(g)

                    den_sb = pb.tile([1, 2 * QB], F32, tag="den", name="den_sb")
                    # custom-DVE recip misreads PSUM at partition base 64;
                    # stage the den row through SBUF first.
                    nc.scalar.copy(out=den_sb[:], in_=o[D:D + 1, :])
                    rd = pb.tile([1, 2 * QB], F32, tag="rd", name="rd")
                    nc.vector.reciprocal_approx_fast(out=rd[:], in_=den_sb[:])
                    rep = mp.tile([P, 2 * QB], F32, tag="mm", name="rep")
                    nc.tensor.matmul(
                        rep[:, 0:QB], lhsT=onesr[:], rhs=rd[0:1, 0:QB],
                        start=True, stop=False,
                    )
                    nc.tensor.matmul(
                        rep[:, QB:2 * QB], lhsT=onesr[:], rhs=rd[0:1, QB:2 * QB],
                        start=False, stop=True,
                    )
                    rep_sb = pb.tile([P, 2 * QB], BF16, tag="repsb", name="rep_sb")
                    nc.vector.tensor_copy(out=rep_sb[:], in_=rep[:])
                    nc.vector.tensor_tensor(
                        out=ons[qb][0:D, :], in0=o[0:D, 0:QB],
                        in1=rep_sb[0:D, 0:QB],
                        op=mybir.AluOpType.mult,
                    )
                    nc.vector.tensor_tensor(
                        out=ons[qb][D:P, :], in0=o[0:D, QB:2 * QB],
                        in1=rep_sb[D:P, QB:2 * QB],
                        op=mybir.AluOpType.mult,
                    )

                    ob = fsb.tile([P, 2 * DIM], BF16, tag="ob", name="ob")
                    for t in range(QB // P):
                        pr = mp.tile([P, DIM], F32, tag="mm", name="pr")
                        nc.tensor.matmul(
                            pr[:],
                            lhsT=ons[qb][:, t * P:(t + 1) * P],
                            rhs=wo_sb[:],
                            start=True, stop=True,
                        )
                        if t == 0:
                            nc.scalar.copy(out=ob[:, 0:DIM], in_=pr[:])
                        else:
                            nc.vector.tensor_copy(out=ob[:, DIM:2 * DIM], in_=pr[:])
                    nc.sync.dma_start(
                        out=out_d[qb * QB:(qb + 1) * QB, :].rearrange(
                            "(t p) d -> p t d", p=P),
                        in_=ob[:].rearrange("p (t d) -> p t d", t=2),
                    )

    nc.compile()
    _CACHE[sched] = nc
    return nc


def _pack(a):
    # [n*128, X] -> [128, n*X]
    n = a.shape[0] // P
    return np.ascontiguousarray(
        a.reshape(n, P, a.shape[1]).transpose(1, 0, 2).reshape(P, -1))


def make_in_maps(x, routes, w_qkv, b_qkv, w_out, order, rk, sched):
    x = np.asarray(x, np.float32)
    w_qkv = np.asarray(w_qkv, np.float32)
    b_qkv = np.asarray(b_qkv, np.float32)
    w_out = np.asarray(w_out, np.float32)

    Cm = np.zeros((S, S), bool)
    Cm[np.arange(S)[:, None], rk] = True          # permuted [q', k']
    cols = []
    for qb, kts in enumerate(sched):
        blk = Cm[qb * QB:(qb + 1) * QB]
        for kt in kts:
            sub = blk[:, kt * P:(kt + 1) * P]     # [256 q, 128 k]
            cols.append(np.where(sub.T, 0.0, MASKNEG))
    msk = np.concatenate(cols, axis=1).astype(NPF8)

    xt = [
        _pack(np.ascontiguousarray(x[b].T[:, order])).astype(NPBF16)
        for b in range(B)
    ]

    in_maps = []
    for core in range(8):
        b = core // 4
        hp = core % 4
        col = hp * P
        wq = _pack(w_qkv[:, col:col + P] * SCALE).astype(NPBF16)
        wk = _pack(w_qkv[:, DIM + col:DIM + col + P]).astype(NPBF16)
        wv = _pack(w_qkv[:, 2 * DIM + col:2 * DIM + col + P]).astype(NPBF16)
        bq = (b_qkv[col:col + P] * SCALE).astype(np.float32).reshape(P, 1)
        wo = np.ascontiguousarray(w_out[col:col + P, :]).astype(NPBF16)
        in_maps.append(dict(
            xt=xt[b], wq=wq, wk=wk, wv=wv, bq=bq, msk=msk, wo=wo,
        ))
    return in_maps


def run(inputs, trace=False, trace_cores=None):
    routes = np.asarray(inputs["routes"])
    order, rk, sched = _schedule(routes)
    nc = build_nc(sched)
    in_maps = make_in_maps(
        inputs["x"], routes, inputs["w_qkv"], inputs["b_qkv"],
        inputs["w_out"], order, rk, sched,
    )
    res = run_bass_kernel_spmd(
        nc, in_maps, list(range(8)), trace=trace, trace_cores=trace_cores,
    )
    b_qkv = np.asarray(inputs["b_qkv"], np.float32)
    w_out = np.asarray(inputs["w_out"], np.float32)
    bias = np.asarray(inputs["b_out"], np.float32) + b_qkv[2 * DIM:] @ w_out
    final = np.zeros((B, S, DIM), np.float32)
    for core in range(8):
        final[core // 4][order] += np.asarray(res.results[core]["out"], np.float32)
    final += bias[None, None, :]
    return final, res


def kernel(**inputs):
    final, _ = run(inputs, trace=False)
    return final


# revision 15
# speedup vs baseline: 1.0633x; 1.0297x over previous
"""CantorAttention Trainium2 kernel — block-sparse routed attention.

Problem (hardcoded): B=2, S=2048, DIM=512, H=8 heads, D=64, K=64 routes.
  qkv = x @ w_qkv + b_qkv ; per-head softmax attention over routes[q, :] ;
  out = attn_out @ w_out + b_out.

Strategy (8 cores): shard batch x head-pairs. Core i handles batch i//4 and
heads (2*(i%4), 2*(i%4)+1).

Sparsity exploit: routes are distinct per query (binary mask).  A single
token permutation (iterated sort by mean routed-neighbour index — derived
from the routes alone) makes the [S, S] route mask block-sparse: each
256-query block touches only a few 128-key tiles.  The host builds that
schedule and compiles a kernel specialized to it; attention runs dense only
on the touched (key-tile, query-block) pairs with an additive {0, -192}
mask folded into the score PSUM via an fp8 identity matmul.

Algebraic simplifications:
  - K-bias dropped: (q+bq).(k+bk) differs from (q+bq).k by a per-query
    constant -> cancels in softmax.
  - V-bias folded into the output bias on the host (softmax weights sum
    to 1), so V = x @ wv with no bias and the host adds
    b_out + b_qkv[2*DIM:] @ w_out once.
  - Denominators ride along in the AV matmul via a ones column appended to
    V (output row 64), then one reciprocal + ones-row broadcast matmuls
    replicate 1/den across each head's 64 partitions for normalization.

Hardware notes baked in:
  - Both heads run concurrently in QK via PE row tiling (contraction 64,
    rows 0-63 / 64-127) but their outputs go to SEPARATE PSUM banks —
    mixing two row-tiled matmuls in one bank crashes the device.  Each
    bank gets exactly one start=True (clears whole-bank has_written);
    every later write overwrites-where-unset / accumulates-where-set.
  - The custom-DVE reciprocal misreads PSUM at partition base 64, so the
    denominator row is staged through SBUF first.
  - Tile tracks dependencies per-tile, so inputs are split into many
    small tiles (x^T slabs, per-chunk q^T/k^T, per-key-tile V, per-group
    masks, per-block outputs) to let phases overlap.
"""

import numpy as np
import ml_dtypes

import concourse.bass as bass
import concourse.bacc as bacc
import concourse.mybir as mybir
import concourse.tile as tile
from concourse.bass_utils import run_bass_kernel_spmd
from concourse.masks import make_identity

BF16 = mybir.dt.bfloat16
F32 = mybir.dt.float32
F8 = mybir.dt.float8e4
NPBF16 = ml_dtypes.bfloat16
NPF8 = ml_dtypes.float8_e4m3

B = 2
S = 2048
DIM = 512
H = 8
D = 64
KR = 64
SCALE = 0.125

P = 128
NC4 = DIM // P    # 4 contraction chunks
QC = 512          # projection slab width
QB = 256          # query block
NQB = S // QB     # 8 query blocks
NKT = S // P      # 16 key tiles
VW = D + 1        # v tile width incl ones column
MASKNEG = -192.0  # exact in fp8 e4m3 (TRN max normal 240); exp(-192) == 0

_CACHE = {}


def _token_order(routes):
    """Permutation clustering tokens so each query block touches few key
    tiles.  Iterated argsort by mean routed-neighbour position; generic
    (no Cantor assumption)."""
    n = routes.shape[0]
    order = np.argsort(routes.mean(axis=1), kind="stable")
    for _ in range(3):
        inv = np.empty(n, np.int64)
        inv[order] = np.arange(n)
        m = inv[routes].mean(axis=1)
        order = order[np.argsort(m[order], kind="stable")]
    return order


def _schedule(routes):
    routes = np.asarray(routes, np.int64)
    order = _token_order(routes)
    inv = np.empty(S, np.int64)
    inv[order] = np.arange(S)
    rk = inv[routes][order]          # [S, K] both sides permuted
    tiles = rk // P
    sched = tuple(
        tuple(sorted(set(tiles[qb * QB:(qb + 1) * QB].ravel().tolist())))
        for qb in range(NQB)
    )
    return order, rk, sched


def build_nc(sched):
    if sched in _CACHE:
        return _CACHE[sched]
    npair = sum(len(k) for k in sched)
    nc = bacc.Bacc(
        "TRN2",
        target_bir_lowering=False,
        debug=False,
        num_devices=8,
    )

    xt_d = nc.dram_tensor("xt", [P, NC4 * S], BF16, kind="ExternalInput").ap()
    wq_d = nc.dram_tensor("wq", [P, NC4 * P], BF16, kind="ExternalInput").ap()
    wk_d = nc.dram_tensor("wk", [P, NC4 * P], BF16, kind="ExternalInput").ap()
    wv_d = nc.dram_tensor("wv", [P, NC4 * P], BF16, kind="ExternalInput").ap()
    bq_d = nc.dram_tensor("bq", [P, 1], F32, kind="ExternalInput").ap()
    msk_d = nc.dram_tensor("msk", [P, npair * QB], F8, kind="ExternalInput").ap()
    wo_d = nc.dram_tensor("wo", [P, DIM], BF16, kind="ExternalInput").ap()
    out_d = nc.dram_tensor("out", [S, DIM], BF16, kind="ExternalOutput").ap()

    qb_off = []
    off = 0
    for kts in sched:
        qb_off.append(off)
        off += len(kts) * QB

    # V tiles ordered by first use across query blocks
    v_order = []
    for kts in sched:
        for kt in kts:
            if kt not in v_order:
                v_order.append(kt)
    for kt in range(NKT):
        if kt not in v_order:
            v_order.append(kt)

    # mask tile DMA grouping
    mgrp = [list(range(0, 3)), list(range(3, 6)), list(range(6, NQB))]

    with tile.TileContext(nc) as tc:
        with tc.tile_pool(name="persist", bufs=1) as pp:
            ident = pp.tile([P, P], BF16, tag="ident")
            make_identity(nc, ident[:])
            idf8 = pp.tile([P, P], F8, tag="idf8")
            nc.scalar.copy(out=idf8[:], in_=ident[:])
            onesr = pp.tile([1, P], F32, tag="onesr")
            nc.vector.memset(onesr[:], 1.0)

            # --- input DMAs, consumption-ordered, fine-grained tiles ---
            w_sb = {}
            for name in ("k", "q", "v"):
                w_sb[name] = pp.tile([P, NC4 * P], BF16, tag=f"w{name}",
                                     name=f"w{name}_sb")
            xtc = [[pp.tile([P, QC], BF16, tag=f"xt{qc}_{c}",
                            name=f"xt{qc}_{c}") for c in range(NC4)]
                   for qc in range(NC4)]
            bq_sb = pp.tile([P, 1], F32, tag="bq")
            wo_sb = pp.tile([P, DIM], BF16, tag="wo")
            msk_sb = {}
            for gi, grp in enumerate(mgrp):
                w = sum(len(sched[qb]) for qb in grp) * QB
                msk_sb[gi] = pp.tile([P, w], F8, tag=f"msk{gi}",
                                     name=f"msk{gi}")

            def xt_load(qc, c):
                nc.sync.dma_start(
                    out=xtc[qc][c][:],
                    in_=xt_d[:, c * S + qc * QC: c * S + (qc + 1) * QC])

            nc.sync.dma_start(out=w_sb["k"][:], in_=wk_d[:, :])
            for c in range(NC4):
                xt_load(0, c)
            nc.sync.dma_start(out=w_sb["q"][:], in_=wq_d[:, :])
            nc.sync.dma_start(
                out=msk_sb[0][:],
                in_=msk_d[:, qb_off[0]:qb_off[0] + msk_sb[0].shape[1]])
            for c in range(NC4):
                xt_load(1, c)
            nc.sync.dma_start(out=w_sb["v"][:], in_=wv_d[:, :])
            nc.sync.dma_start(out=bq_sb[:], in_=bq_d[:, :])
            for c in range(NC4):
                xt_load(2, c)
            nc.sync.dma_start(out=wo_sb[:], in_=wo_d[:, :])
            for c in range(NC4):
                xt_load(3, c)
            for gi in (1, 2):
                o0 = qb_off[mgrp[gi][0]]
                nc.gpsimd.dma_start(
                    out=msk_sb[gi][:], in_=msk_d[:, o0:o0 + msk_sb[gi].shape[1]])

            def mslice(qb, pi):
                gi = 0 if qb < 3 else (1 if qb < 6 else 2)
                o0 = qb_off[qb] - qb_off[mgrp[gi][0]] + pi * QB
                return msk_sb[gi][:, o0:o0 + QB]

            kTs = [pp.tile([P, QC], BF16, tag=f"kT{i}", name=f"kT{i}")
                   for i in range(NC4)]
            qTs = [pp.tile([P, QC], BF16, tag=f"qT{i}", name=f"qT{i}")
                   for i in range(NC4)]
            v0t = {kt: pp.tile([P, VW], BF16, tag=f"v0_{kt}", name=f"v0_{kt}")
                   for kt in range(NKT)}
            v1t = {kt: pp.tile([P, VW], BF16, tag=f"v1_{kt}", name=f"v1_{kt}")
                   for kt in range(NKT)}
            for kt in range(NKT):
                nc.vector.memset(v0t[kt][:, D:VW], 1.0)
                nc.vector.memset(v1t[kt][:, D:VW], 1.0)
            ons = [pp.tile([P, QB], BF16, tag=f"on{qb}", name=f"on{qb}")
                   for qb in range(NQB)]

            def kslice(kt, rows):
                return kTs[kt // NC4][rows, (kt % NC4) * P:(kt % NC4 + 1) * P]

            def qslice(qb, rows):
                return qTs[qb // 2][rows, (qb % 2) * QB:(qb % 2 + 1) * QB]

            # ---- Phase A: projections ----
            with tc.tile_pool(name="pa", bufs=3, space="PSUM") as pa:
                for qc in range(NC4):          # kT = Wk^T X^T  (no bias)
                    ps = pa.tile([P, QC], F32, tag="qkps", name="kps")
                    for c in range(NC4):
                        nc.tensor.matmul(
                            ps[:],
                            lhsT=w_sb["k"][:, c * P:(c + 1) * P],
                            rhs=xtc[qc][c][:],
                            start=(c == 0),
                            stop=(c == NC4 - 1),
                        )
                    if qc % 2 == 0:
                        nc.scalar.copy(out=kTs[qc][:], in_=ps[:])
                    else:
                        nc.vector.tensor_copy(out=kTs[qc][:], in_=ps[:])
                for qc in range(NC4):          # qT = Wq'^T X^T + bq' (scaled)
                    ps = pa.tile([P, QC], F32, tag="qkps", name="qps")
                    for c in range(NC4):
                        nc.tensor.matmul(
                            ps[:],
                            lhsT=w_sb["q"][:, c * P:(c + 1) * P],
                            rhs=xtc[qc][c][:],
                            start=(c == 0),
                            stop=(c == NC4 - 1),
                        )
                    nc.vector.tensor_tensor(
                        out=qTs[qc][:],
                        in0=ps[:],
                        in1=bq_sb[:].to_broadcast([P, QC]),
                        op=mybir.AluOpType.add,
                    )
                for vi, kt in enumerate(v_order):   # V direct [tokens, dims]
                    vps = pa.tile([P, P], F32, tag="vps", name="vps")
                    for c in range(NC4):
                        nc.tensor.matmul(
                            vps[:],
                            lhsT=xtc[kt // NC4][c][:, (kt % NC4) * P:
                                                   (kt % NC4 + 1) * P],
                            rhs=w_sb["v"][:, c * P:(c + 1) * P],
                            start=(c == 0),
                            stop=(c == NC4 - 1),
                        )
                    if vi % 2 == 0:
                        nc.vector.tensor_copy(out=v0t[kt][:, 0:D], in_=vps[:, 0:D])
                        nc.scalar.copy(out=v1t[kt][:, 0:D], in_=vps[:, D:2 * D])
                    else:
                        nc.scalar.copy(out=v0t[kt][:, 0:D], in_=vps[:, 0:D])
                        nc.vector.tensor_copy(out=v1t[kt][:, 0:D], in_=vps[:, D:2 * D])

            # ---- Phase B: block-sparse attention + normalize + project ----
            with tc.tile_pool(name="ps_s", bufs=2, space="PSUM") as sp, \
                 tc.tile_pool(name="ps_o", bufs=2, space="PSUM") as opool, \
                 tc.tile_pool(name="ps_m", bufs=2, space="PSUM") as mp, \
                 tc.tile_pool(name="pb", bufs=3) as pb, \
                 tc.tile_pool(name="fin", bufs=3) as fsb:
                for qb in range(NQB):
                    kts = sched[qb]
                    nkt = len(kts)
                    o = opool.tile([P, 2 * QB], F32, tag="o", name="o")
                    groups = [kts[i:i + 2] for i in range(0, nkt, 2)]
                    pi = 0
                    for g in groups:
                        # s spans 2 banks: bank A (cols 0:2QB) holds the
                        # group's h0 scores, bank B (2QB:4QB) holds h1.
                        ng = len(g)
                        s = sp.tile([P, 4 * QB], F32, tag="s", name="s")
                        pmt = pb.tile([P, 4 * QB], BF16, tag="pm", name="pm")
                        for jj, kt in enumerate(g):
                            a0 = jj * QB              # h0 region (bank A)
                            a1 = 2 * QB + jj * QB     # h1 region (bank B)
                            nc.tensor.matmul(
                                s[:, a0:a0 + QB],
                                lhsT=kslice(kt, slice(0, D)),
                                rhs=qslice(qb, slice(0, D)),
                                start=(jj == 0), stop=False,
                            )
                            nc.tensor.matmul(
                                s[:, a1:a1 + QB],
                                lhsT=kslice(kt, slice(D, P)),
                                rhs=qslice(qb, slice(D, P)),
                                start=(jj == 0), stop=False,
                            )
                            nc.tensor.matmul(
                                s[:, a0:a0 + QB],
                                lhsT=idf8[:],
                                rhs=mslice(qb, pi + jj),
                                start=False, stop=(jj == ng - 1),
                            )
                            nc.tensor.matmul(
                                s[:, a1:a1 + QB],
                                lhsT=idf8[:],
                                rhs=mslice(qb, pi + jj),
                                start=False, stop=(jj == ng - 1),
                            )
                        if ng == 2:
                            nc.scalar.activation(
                                pmt[:], s[:], mybir.ActivationFunctionType.Exp
                            )
                        else:
                            nc.scalar.activation(
                                pmt[:, 0:QB], s[:, 0:QB],
                                mybir.ActivationFunctionType.Exp
                            )
                            nc.scalar.activation(
                                pmt[:, 2 * QB:3 * QB], s[:, 2 * QB:3 * QB],
                                mybir.ActivationFunctionType.Exp
                            )
                        for jj, kt in enumerate(g):
                            j = pi + jj
                            a0 = jj * QB
                            a1 = 2 * QB + jj * QB
                            nc.tensor.matmul(
                                o[0:VW, 0:QB],
                                lhsT=v0t[kt][:],
                                rhs=pmt[:, a0:a0 + QB],
                                start=(j == 0), stop=False,
                            )
                            nc.tensor.matmul(
                                o[0:VW, QB:2 * QB],
                                lhsT=v1t[kt][:],
                                rhs=pmt[:, a1:a1 + QB],
                                start=False, stop=(j == nkt - 1),
                            )
                        pi += len(g)

                    den_sb = pb.tile([1, 2 * QB], F32, tag="den", name="den_sb")
                    # custom-DVE recip misreads PSUM at partition base 64;
                    # stage the den row through SBUF first.
                    nc.scalar.copy(out=den_sb[:], in_=o[D:D + 1, :])
                    rd = pb.tile([1, 2 * QB], F32, tag="rd", name="rd")
                    nc.vector.reciprocal_approx_fast(out=rd[:], in_=den_sb[:])
                    rep = mp.tile([P, 2 * QB], F32, tag="mm", name="rep")
                    nc.tensor.matmul(
                        rep[:, 0:QB], lhsT=onesr[:], rhs=rd[0:1, 0:QB],
                        start=True, stop=False,
                    )
                    nc.tensor.matmul(
                        rep[:, QB:2 * QB], lhsT=onesr[:], rhs=rd[0:1, QB:2 * QB],
                        start=False, stop=True,
                    )
                    rep_sb = pb.tile([P, 2 * QB], BF16, tag="repsb", name="rep_sb")
                    nc.vector.tensor_copy(out=rep_sb[:], in_=rep[:])
                    nc.vector.tensor_tensor(
                        out=ons[qb][0:D, :], in0=o[0:D, 0:QB],
                        in1=rep_sb[0:D, 0:QB],
                        op=mybir.AluOpType.mult,
                    )
                    nc.vector.tensor_tensor(
                        out=ons[qb][D:P, :], in0=o[0:D, QB:2 * QB],
                        in1=rep_sb[D:P, QB:2 * QB],
                        op=mybir.AluOpType.mult,
                    )

                    ob = fsb.tile([P, 2 * DIM], BF16, tag="ob", name="ob")
                    for t in range(QB // P):
                        pr = mp.tile([P, DIM], F32, tag="mm", name="pr")
                        nc.tensor.matmul(
                            pr[:],
                            lhsT=ons[qb][:, t * P:(t + 1) * P],
                            rhs=wo_sb[:],
                            start=True, stop=True,
                        )
                        if t == 0:
                            nc.scalar.copy(out=ob[:, 0:DIM], in_=pr[:])
                        else:
                            nc.vector.tensor_copy(out=ob[:, DIM:2 * DIM], in_=pr[:])
                    nc.sync.dma_start(
                        out=out_d[qb * QB:(qb + 1) * QB, :].rearrange(
                            "(t p) d -> p t d", p=P),
                        in_=ob[:].rearrange("p (t d) -> p t d", t=2),
                    )

    nc.compile()
    _CACHE[sched] = nc
    return nc


def _pack(a):
    # [n*128, X] -> [128, n*X]
    n = a.shape[0] // P
    return np.ascontiguousarray(
        a.reshape(n, P, a.shape[1]).transpose(1, 0, 2).reshape(P, -1))


def make_in_maps(x, routes, w_qkv, b_qkv, w_out, order, rk, sched):
    x = np.asarray(x, np.float32)
    w_qkv = np.asarray(w_qkv, np.float32)
    b_qkv = np.asarray(b_qkv, np.float32)
    w_out = np.asarray(w_out, np.float32)

    Cm = np.zeros((S, S), bool)
    Cm[np.arange(S)[:, None], rk] = True          # permuted [q', k']
    cols = []
    for qb, kts in enumerate(sched):
        blk = Cm[qb * QB:(qb + 1) * QB]
        for kt in kts:
            sub = blk[:, kt * P:(kt + 1) * P]     # [256 q, 128 k]
            cols.append(np.where(sub.T, 0.0, MASKNEG))
    msk = np.concatenate(cols, axis=1).astype(NPF8)

    xt = [
        _pack(np.ascontiguousarray(x[b].T[:, order])).astype(NPBF16)
        for b in range(B)
    ]

    in_maps = []
    for core in range(8):
        b = core // 4
        hp = core % 4
        col = hp * P
        wq = _pack(w_qkv[:, col:col + P] * SCALE).astype(NPBF16)
        wk = _pack(w_qkv[:, DIM + col:DIM + col + P]).astype(NPBF16)
        wv = _pack(w_qkv[:, 2 * DIM + col:2 * DIM + col + P]).astype(NPBF16)
        bq = (b_qkv[col:col + P] * SCALE).astype(np.float32).reshape(P, 1)
        wo = np.ascontiguousarray(w_out[col:col + P, :]).astype(NPBF16)
        in_maps.append(dict(
            xt=xt[b], wq=wq, wk=wk, wv=wv, bq=bq, msk=msk, wo=wo,
        ))
    return in_maps


def run(inputs, trace=False, trace_cores=None):
    routes = np.asarray(inputs["routes"])
    order, rk, sched = _schedule(routes)
    nc = build_nc(sched)
    in_maps = make_in_maps(
        inputs["x"], routes, inputs["w_qkv"], inputs["b_qkv"],
        inputs["w_out"], order, rk, sched,
    )
    res = run_bass_kernel_spmd(
        nc, in_maps, list(range(8)), trace=trace, trace_cores=trace_cores,
    )
    b_qkv = np.asarray(inputs["b_qkv"], np.float32)
    w_out = np.asarray(inputs["w_out"], np.float32)
    bias = np.asarray(inputs["b_out"], np.float32) + b_qkv[2 * DIM:] @ w_out
    final = np.zeros((B, S, DIM), np.float32)
    for core in range(8):
        final[core // 4][order] += np.asarray(res.results[core]["out"], np.float32)
    final += bias[None, None, :]
    return final, res


def kernel(**inputs):
    final, _ = run(inputs, trace=False)
    return final
